# revision 1
# baseline (speedup 1.0000x reference)
"""Trainium2 Bass kernel for a single-head causal self-attention variant.

Reference semantics (B=4, S=2048, D=1024):
    q = x @ wq.T ; k = x @ wk.T ; v = x @ wv.T
    scores = q @ k.T / sqrt(D)          # [B, S, S]
    a = softmax(scores, axis=-2)        # softmax over the QUERY axis, per key column
    a = triu(a)                         # keep q <= k, applied AFTER softmax
    out = a.T @ v                       # out row i = sum_{q<=i} a[q,i] * v[q]

Key algebraic folds (single head):
  * scores = x @ (wq.T @ wk) @ x.T, so wq/wk fold into one matrix
    MT = (wk.T @ wq)/sqrt(D) on the host -> no Q projection on device.
  * softmax needs no max subtraction here (scores are O(1) by construction),
    so a column is exp(s) / colsum, and the normalization can be applied to
    the output rows at the very end: out[k] = (sum_q Emask[q,k] v[q]) / colsum[k].

Sharding (8 cores): core = (batch b = core//2, half h = core%2). Each core owns
the interleaved local k-chunks kc_global = 2j+h, j=0..7 (128 columns each) of
its batch; the interleaving balances the triangular A^T V work between the two
halves. Softmax denominators need all 2048 q per column, so each core computes
scores/exp for all q in its 1024 columns; A^T V skips blocks that the causal
mask zeroes entirely.

All matmuls run as float32r (fp32 data, fp22 multiply) with 512-wide free
dims, which streams at 1 column/cycle on the PE like bf16 (fp32r matmuls are
self-loading, so each pays its 128-column weight load; measured ~260 us/core
steady-state vs a ~222 us PE floor). Column sums accumulate in two PSUM banks
across all 16 q-chunks, emitted after each group's A^T V matmuls so the
in-order PE stream never head-of-line blocks on an ACT exp. The device returns
unnormalized U plus the column-sum vector (cso); the O(S*D) elementwise
divide happens in gather() on the host, which lets each finished 128-row
output block DMA out mid-kernel instead of serializing a normalization tail.
A bf16 mode exists (use_bf16) but measured only ~8% faster at 16x worse
error, so fp32r is the default. End-to-end rel-l2 error ~2e-4 vs the fp32
reference.
"""

import numpy as np

B, S, D = 4, 2048, 1024
P = 128
SK = 1024          # k columns per core
KD = D // P        # 8 contraction chunks
NJ = SK // P       # 8 local k chunks
NG = 4             # q groups of 512
NQL = 4            # 128-row q chunks per group
NCORES = 8

_cache = {}
_ABLATE = set()  # test-only: {"phase3","av","cs","exp_ident","phasek"}


def _build_module(reps=1, accum=False, use_bf16=False):
    import concourse.bacc as bacc
    import concourse.tile as tile
    from concourse import mybir

    f32 = mybir.dt.float32
    f32r = mybir.dt.bfloat16 if use_bf16 else mybir.dt.float32r
    dmadt = f32r  # DRAM input dtype for matmul operands
    Exp = mybir.ActivationFunctionType.Exp

    nc = bacc.Bacc("TRN2", target_bir_lowering=False, debug=False,
                   num_devices=NCORES)

    xT = nc.dram_tensor("xT", [D, S], dmadt, kind="ExternalInput").ap()
    xtk = nc.dram_tensor("xtk", [D, SK], dmadt, kind="ExternalInput").ap()
    mt = nc.dram_tensor("mt", [D, D], dmadt, kind="ExternalInput").ap()
    wvT = nc.dram_tensor("wvT", [D, D], dmadt, kind="ExternalInput").ap()
    mask0 = nc.dram_tensor("mask0", [P, P], dmadt, kind="ExternalInput").ap()
    mask1 = nc.dram_tensor("mask1", [P, P], dmadt, kind="ExternalInput").ap()
    onesd = nc.dram_tensor("onesd", [P, 1], dmadt, kind="ExternalInput").ap()
    out = nc.dram_tensor("out", [SK, D], f32, kind="ExternalOutput").ap()
    cso = nc.dram_tensor("cso", [1, SK], f32, kind="ExternalOutput").ap()

    def mm(ps, lhsT, rhs, start, stop):
        nc.tensor.matmul(ps, lhsT, rhs, start=start, stop=stop)

    with tile.TileContext(nc) as tc:
        from contextlib import ExitStack
        for _rep in range(reps):
          with ExitStack() as ctx:
            persist = ctx.enter_context(tc.tile_pool(name="persist", bufs=1))
            psum = ctx.enter_context(tc.tile_pool(name="psum", bufs=2, space="PSUM"))

            ones_t = persist.tile([P, 1], f32r, tag="ones")
            nc.sync.dma_start(ones_t, onesd if use_bf16 else onesd.bitcast(f32r))
            m0_t = persist.tile([P, P], f32r, tag="m0")
            nc.sync.dma_start(m0_t, mask0 if use_bf16 else mask0.bitcast(f32r))
            m1_t = persist.tile([P, P], f32r, tag="m1")
            nc.sync.dma_start(m1_t, mask1 if use_bf16 else mask1.bitcast(f32r))

            wv_t = persist.tile([P, KD, D], f32r, tag="wv")
            for c in range(KD):
                nc.sync.dma_start(wv_t[:, c, :], (wvT if use_bf16 else wvT.bitcast(f32r))[c * P:(c + 1) * P, :])

            km_t = persist.tile([P, KD, SK], f32r, tag="km")
            u = [persist.tile([P, D], f32, tag=f"u{j}", name=f"u{j}")
                 for j in range(NJ)]

            # ---- phase K: KM[dq, k] = sum_dk M[dq,dk] * x_k^T[dk, k] ----
            with tc.tile_pool(name="pk", bufs=1) as pk:
                mt_t = pk.tile([P, KD, D], f32r, tag="mt")
                xtk_t = pk.tile([P, KD, SK], f32r, tag="xtk")
                for c in range(KD):
                    nc.sync.dma_start(mt_t[:, c, :], (mt if use_bf16 else mt.bitcast(f32r))[c * P:(c + 1) * P, :])
                    nc.sync.dma_start(xtk_t[:, c, :], (xtk if use_bf16 else xtk.bitcast(f32r))[c * P:(c + 1) * P, :])
                for dq in range(0 if "phasek" in _ABLATE else KD):
                    for kf in range(2):
                        ps = psum.tile([P, 512], f32, tag="ps_mm", name="ps_km", bufs=5)
                        for c in range(KD):
                            mm(ps, mt_t[:, c, dq * P:(dq + 1) * P],
                               xtk_t[:, c, kf * 512:(kf + 1) * 512],
                               start=(c == 0), stop=(c == KD - 1))
                        nc.vector.tensor_copy(km_t[:, dq, kf * 512:(kf + 1) * 512], ps)

            # ---- phase 2: stream q in 4 groups of 512 ----
            cs_ps = [psum.tile([1, 512], f32, tag=f"ps_cs{kf}", name=f"ps_cs{kf}",
                               bufs=1) for kf in range(2)]
            qgp = ctx.enter_context(tc.tile_pool(name="qgp", bufs=2))
            vegp = ctx.enter_context(tc.tile_pool(name="vegp", bufs=2))
            for g in range(NG):
                xg = qgp.tile([P, KD, 512], f32r, tag="xg", name=f"xg{g}")
                for c in range(KD):
                    nc.sync.dma_start(
                        xg[:, c, :],
                        (xT if use_bf16 else xT.bitcast(f32r))[c * P:(c + 1) * P, g * 512:(g + 1) * 512])
                eg, vg = [], []
                for ql in range(NQL):
                    # V[q, dv] for this 128-row q chunk
                    vt = vegp.tile([P, D], f32r, tag=f"v{ql}", name=f"v{g}_{ql}")
                    for dv in range(2):
                        ps = psum.tile([P, 512], f32, tag="ps_mm", name="ps_v", bufs=5)
                        for c in range(KD):
                            mm(ps, xg[:, c, ql * P:(ql + 1) * P],
                               wv_t[:, c, dv * 512:(dv + 1) * 512],
                               start=(c == 0), stop=(c == KD - 1))
                        nc.vector.tensor_copy(vt[:, dv * 512:(dv + 1) * 512], ps)
                    vg.append(vt)
                    # E[q, k] = exp(scores) for this q chunk x all local k
                    et = vegp.tile([P, SK], f32r, tag=f"e{ql}", name=f"e{g}_{ql}")
                    for kf in range(2):
                        ps = psum.tile([P, 512], f32, tag="ps_mm", name="ps_e", bufs=5)
                        for c in range(KD):
                            mm(ps, xg[:, c, ql * P:(ql + 1) * P],
                               km_t[:, c, kf * 512:(kf + 1) * 512],
                               start=(c == 0), stop=(c == KD - 1))
                        nc.scalar.activation(et[:, kf * 512:(kf + 1) * 512], ps, Exp)
                    eg.append(et)
                # causal mask: the j == qc//2 block is multiplied into a
                # separate tile (keeps eg read-only, so colsum and AV don't
                # serialize on a WAR hazard); blocks j > qc//2 are all-ones,
                # blocks j < qc//2 are never read by AV.
                emask = []
                for ql in range(NQL):
                    qc = g * NQL + ql
                    jm = qc // 2
                    mk = m0_t if qc % 2 == 0 else m1_t
                    em = vegp.tile([P, P], f32r, tag=f"em{ql}", name=f"em{g}_{ql}")
                    nc.vector.tensor_mul(em, eg[ql][:, jm * P:(jm + 1) * P], mk)
                    emask.append(em)
                # U[j] += Emask[qchunk]^T V[qchunk] for valid blocks (qc <= 2j+1)
                for j in range(() if "av" in _ABLATE else range(2 * g, NJ)) if False else (range(0) if "av" in _ABLATE else range(2 * g, NJ)):
                    hi = min(NQL - 1, 2 * j + 1 - 4 * g)
                    for dv in range(2):
                        ps = psum.tile([P, 512], f32, tag="ps_av", name="ps_av", bufs=1)
                        for ql in range(hi + 1):
                            qc = g * NQL + ql
                            lhs = emask[ql] if j == qc // 2 else \
                                eg[ql][:, j * P:(j + 1) * P]
                            mm(ps, lhs,
                               vg[ql][:, dv * 512:(dv + 1) * 512],
                               start=(ql == 0), stop=(ql == hi))
                        sl = u[j][:, dv * 512:(dv + 1) * 512]
                        if g == 0:
                            nc.vector.tensor_copy(sl, ps)
                        else:
                            nc.vector.tensor_add(sl, sl, ps)
                        if g == min(NG - 1, (2 * j + 1) // NQL):
                            # last contribution to u[j]: ship it now so the
                            # output DMA overlaps the remaining groups
                            dst = out[j * P:(j + 1) * P, dv * 512:(dv + 1) * 512]
                            if accum:
                                nc.gpsimd.dma_start(dst, sl,
                                                    accum_op=mybir.AluOpType.add)
                            else:
                                nc.sync.dma_start(dst, sl)
                # column sums: one psum accumulation chain per kf across ALL
                # 16 q chunks (emitted after AV so the in-order PE stream never
                # stalls waiting for an exp to finish)
                if "cs" not in _ABLATE:
                    for kf in range(2):
                        for ql in range(NQL):
                            qc = g * NQL + ql
                            nc.tensor.matmul(
                                cs_ps[kf], ones_t,
                                eg[ql][:, kf * 512:(kf + 1) * 512],
                                start=(qc == 0), stop=(qc == NG * NQL - 1),
                                skip_group_check=True)

            # ---- epilogue: ship column sums; normalization happens on host ----
            for kf in range(2):
                cs_sb = persist.tile([1, 512], f32, tag=f"cs_sb{kf}",
                                     name=f"cs_sb{kf}")
                nc.vector.tensor_copy(cs_sb, cs_ps[kf])
                dst = cso[:, kf * 512:(kf + 1) * 512]
                if accum:
                    nc.gpsimd.dma_start(dst, cs_sb, accum_op=mybir.AluOpType.add)
                else:
                    nc.sync.dma_start(dst, cs_sb)

    nc.compile()
    return nc


def _get_nc(reps=1, accum=False, use_bf16=False):
    key = ("nc", reps, accum, use_bf16)
    if key not in _cache:
        _cache[key] = _build_module(reps, accum, use_bf16)
    return _cache[key]


def make_in_maps(x, wq, wk, wv, use_bf16=False):
    x = np.asarray(x, np.float32)
    mt = ((np.asarray(wk, np.float64).T @ np.asarray(wq, np.float64))
          / np.sqrt(float(D))).astype(np.float32)
    wvT = np.ascontiguousarray(np.asarray(wv, np.float32).T)
    tri = np.triu(np.ones((P, P), np.float32))
    masks = {
        0: (tri, np.zeros((P, P), np.float32)),          # h=0: diag block, zero block
        1: (np.ones((P, P), np.float32), tri),           # h=1: all-ones block, diag block
    }
    in_maps = []
    for core in range(NCORES):
        b, h = core // 2, core % 2
        xTb = np.ascontiguousarray(x[b].T)               # [D, S]
        cols = np.concatenate(
            [np.arange((2 * j + h) * P, (2 * j + h + 1) * P) for j in range(NJ)])
        xtk = np.ascontiguousarray(xTb[:, cols])         # [D, SK]
        m0, m1 = masks[h]
        m = {
            "xT": xTb, "xtk": xtk, "mt": mt, "wvT": wvT,
            "mask0": m0, "mask1": m1, "onesd": np.ones((P, 1), np.float32),
        }
        if use_bf16:
            import ml_dtypes
            m = {k: v.astype(ml_dtypes.bfloat16) for k, v in m.items()}
        in_maps.append(m)
    return in_maps


def gather(results):
    full = np.empty((B, S, D), np.float32)
    for core in range(NCORES):
        b, h = core // 2, core % 2
        o = results[core]["out"] / results[core]["cso"][0][:, None]
        for j in range(NJ):
            full[b, (2 * j + h) * P:(2 * j + h + 1) * P, :] = \
                o[j * P:(j + 1) * P, :]
    return full


def kernel(x, wq, wk, wv):
    from concourse.bass_utils import run_bass_kernel_spmd
    nc = _get_nc()
    in_maps = make_in_maps(x, wq, wk, wv)
    res = run_bass_kernel_spmd(nc, in_maps, core_ids=list(range(NCORES)))
    return gather(res.results)



# revision 3
# speedup vs baseline: 1.9623x; 1.9623x over previous
"""Trainium2 Bass kernel for a single-head causal self-attention variant.

Reference semantics (B=4, S=2048, D=1024):
    q = x @ wq.T ; k = x @ wk.T ; v = x @ wv.T
    scores = q @ k.T / sqrt(D)          # [B, S, S]
    a = softmax(scores, axis=-2)        # softmax over the QUERY axis, per key column
    a = triu(a)                         # keep q <= k, applied AFTER softmax
    out = a.T @ v                       # out row i = sum_{q<=i} a[q,i] * v[q]

Algebraic folds (single head):
  * scores = x @ (wq.T @ wk) @ x.T, so wq/wk fold into MT = (wk.T @ wq)/sqrt(D)
    on the host -> no Q projection on device.
  * no max subtraction needed (scores are O(1)); normalization by the column
    sum is applied on the host to the unnormalized output U and colsum vector.

Sharding (8 cores): core = (batch b = core//2, half h = core%2). Each core owns
interleaved k-chunks kc_global = 2j+h, j=0..7 (128 cols each) of its batch.

This version runs every matmul in bf16 (512-wide moving operands; the walrus
ISA check rejects 1024-wide PSUM outputs). Unlike float32r (self-loading:
each matmul serializes its own 128-col weight load), bf16 gets
compiler-automatic Fast Weight Load with LDWEIGHTS hoisted ahead of in-flight
matmuls by the PE's 64-deep reorder window, hiding the weight-load bubbles:
784 big matmuls x 512 cols at 2.4 GHz ~ 167us PE floor vs the fp32r
version's ~222us (which pays an exposed 128-cycle load per matmul).

Structure per core: phase K computes KM = MT.T @ xk^T (64 MMs); phase B
computes V[q,:] and E=exp(scores) for all 16 q-chunks, holding ALL E and V
tiles in SBUF (bf16, 64KB/partition); phase C runs one long PSUM accumulation
chain per output k-chunk j over the valid q-chunks (72 MMs), so U never
round-trips through SBUF adds. Column sums ride the PE as thin ones-matmuls,
one chain across all 16 chunks, each emitted one chunk late so the in-order PE
stream never waits on the ACT exp. The device returns unnormalized U and the
column sums (cso); the final divide happens in gather() on the host, letting
finished output blocks DMA out mid-kernel. End-to-end rel-l2 error ~3e-3 vs
the fp32 reference (bf16 rounding), well inside the 2e-2 gate.
"""

import numpy as np

B, S, D = 4, 2048, 1024
P = 128
SK = 1024          # k columns per core
KD = D // P        # 8 contraction chunks
NJ = SK // P       # 8 local k chunks
NG = 4             # q groups of 512
NQL = 4            # 128-row q chunks per group
NQC = NG * NQL     # 16 q chunks
NCORES = 8

_cache = {}


def _build_module(reps=1, accum=False):
    import concourse.bacc as bacc
    import concourse.tile as tile
    from concourse import mybir

    f32 = mybir.dt.float32
    bf16 = mybir.dt.bfloat16
    Exp = mybir.ActivationFunctionType.Exp

    nc = bacc.Bacc("TRN2", target_bir_lowering=False, debug=False,
                   num_devices=NCORES)

    xT = nc.dram_tensor("xT", [D, S], bf16, kind="ExternalInput").ap()
    xtk = nc.dram_tensor("xtk", [D, SK], bf16, kind="ExternalInput").ap()
    mt = nc.dram_tensor("mt", [D, D], bf16, kind="ExternalInput").ap()
    wvT = nc.dram_tensor("wvT", [D, D], bf16, kind="ExternalInput").ap()
    mask0 = nc.dram_tensor("mask0", [P, P], bf16, kind="ExternalInput").ap()
    mask1 = nc.dram_tensor("mask1", [P, P], bf16, kind="ExternalInput").ap()
    onesd = nc.dram_tensor("onesd", [P, 1], bf16, kind="ExternalInput").ap()
    out = nc.dram_tensor("out", [SK, D], f32, kind="ExternalOutput").ap()
    cso = nc.dram_tensor("cso", [1, SK], f32, kind="ExternalOutput").ap()

    mm = nc.tensor.matmul

    with tile.TileContext(nc) as tc:
        from contextlib import ExitStack
        for _rep in range(reps):
          with ExitStack() as ctx:
            persist = ctx.enter_context(tc.tile_pool(name="persist", bufs=1))
            psum = ctx.enter_context(tc.tile_pool(name="psum", bufs=1,
                                                  space="PSUM"))

            ones_t = persist.tile([P, 1], bf16, tag="ones")
            nc.sync.dma_start(ones_t, onesd)
            m0_t = persist.tile([P, P], bf16, tag="m0")
            nc.sync.dma_start(m0_t, mask0)
            m1_t = persist.tile([P, P], bf16, tag="m1")
            nc.sync.dma_start(m1_t, mask1)

            wv_t = persist.tile([P, KD, D], bf16, tag="wv")
            for c in range(KD):
                nc.sync.dma_start(wv_t[:, c, :], wvT[c * P:(c + 1) * P, :])

            km_t = persist.tile([P, KD, SK], bf16, tag="km")
            eg_t = persist.tile([P, NQC, SK], bf16, tag="eg")
            vg_t = persist.tile([P, NQC, D], bf16, tag="vg")
            em_t = persist.tile([P, NQC, P], bf16, tag="em")

            # ---- phase K: KM[dq, k] = sum_dk MT[dk, dq] * x_k^T[dk, k] ----
            with tc.tile_pool(name="pk", bufs=1) as pk:
                mt_t = pk.tile([P, KD, D], bf16, tag="mt")
                xtk_t = pk.tile([P, KD, SK], bf16, tag="xtk")
                for c in range(KD):
                    nc.sync.dma_start(mt_t[:, c, :], mt[c * P:(c + 1) * P, :])
                    nc.sync.dma_start(xtk_t[:, c, :], xtk[c * P:(c + 1) * P, :])
                for dq in range(KD):
                    for kf in range(2):
                        ps = psum.tile([P, 512], f32, tag="ps_mm",
                                       name="ps_km", bufs=6)
                        for c in range(KD):
                            mm(ps, mt_t[:, c, dq * P:(dq + 1) * P],
                               xtk_t[:, c, kf * 512:(kf + 1) * 512],
                               start=(c == 0), stop=(c == KD - 1))
                        nc.vector.tensor_copy(
                            km_t[:, dq, kf * 512:(kf + 1) * 512], ps)

            # ---- phase B: V and E for all 16 q chunks; cs rides one late ----
            cs_ps = [psum.tile([1, 512], f32, tag=f"ps_cs{kf}",
                                name=f"ps_cs{kf}", bufs=1) for kf in range(2)]
            qgp = ctx.enter_context(tc.tile_pool(name="qgp", bufs=2))
            for g in range(NG):
                xg = qgp.tile([P, KD, 512], bf16, tag="xg", name=f"xg{g}")
                for c in range(KD):
                    nc.sync.dma_start(
                        xg[:, c, :], xT[c * P:(c + 1) * P, g * 512:(g + 1) * 512])
                for ql in range(NQL):
                    qc = g * NQL + ql
                    # V[q, dv] for this 128-row q chunk
                    for dv in range(2):
                        ps_v = psum.tile([P, 512], f32, tag="ps_mm",
                                         name="ps_v", bufs=6)
                        for c in range(KD):
                            mm(ps_v, xg[:, c, ql * P:(ql + 1) * P],
                               wv_t[:, c, dv * 512:(dv + 1) * 512],
                               start=(c == 0), stop=(c == KD - 1))
                        nc.vector.tensor_copy(
                            vg_t[:, qc, dv * 512:(dv + 1) * 512], ps_v)
                    # E[q, k] = exp(scores) for this q chunk x all local k
                    for kf in range(2):
                        ps_e = psum.tile([P, 512], f32, tag="ps_mm",
                                         name="ps_e", bufs=6)
                        for c in range(KD):
                            mm(ps_e, xg[:, c, ql * P:(ql + 1) * P],
                               km_t[:, c, kf * 512:(kf + 1) * 512],
                               start=(c == 0), stop=(c == KD - 1))
                        nc.scalar.activation(
                            eg_t[:, qc, kf * 512:(kf + 1) * 512], ps_e, Exp)
                    # diagonal-block causal mask on DVE (h=0: tri/zero masks,
                    # h=1: ones/tri — one NEFF serves both halves)
                    jm = qc // 2
                    mk = m0_t if qc % 2 == 0 else m1_t
                    nc.vector.tensor_mul(em_t[:, qc, :],
                                         eg_t[:, qc, jm * P:(jm + 1) * P], mk)
                    # column-sum chains, one chunk behind the exp producer
                    if qc > 0:
                        for kf in range(2):
                            mm(cs_ps[kf], ones_t,
                               eg_t[:, qc - 1, kf * 512:(kf + 1) * 512],
                               start=(qc == 1), stop=False,
                               skip_group_check=True)
            for kf in range(2):
                mm(cs_ps[kf], ones_t,
                   eg_t[:, NQC - 1, kf * 512:(kf + 1) * 512],
                   start=False, stop=True, skip_group_check=True)
            cs_sb = persist.tile([1, SK], f32, tag="cs_sb")
            for kf in range(2):
                nc.vector.tensor_copy(cs_sb[:, kf * 512:(kf + 1) * 512],
                                      cs_ps[kf])
            if accum:
                nc.gpsimd.dma_start(cso, cs_sb, accum_op=mybir.AluOpType.add)
            else:
                nc.sync.dma_start(cso, cs_sb)

            # ---- phase C: U[j] = sum_{qc<=2j+1} Emask[qc]^T V[qc] ----
            ogp = ctx.enter_context(tc.tile_pool(name="ogp", bufs=2))
            for j in range(NJ):
                hi = 2 * j + 1
                for dv in range(2):
                    ps_av = psum.tile([P, 512], f32, tag="ps_mm",
                                      name="ps_av", bufs=6)
                    for qc in range(hi + 1):
                        lhs = em_t[:, qc, :] if qc // 2 == j else \
                            eg_t[:, qc, j * P:(j + 1) * P]
                        mm(ps_av, lhs, vg_t[:, qc, dv * 512:(dv + 1) * 512],
                           start=(qc == 0), stop=(qc == hi))
                    u_sb = ogp.tile([P, 512], f32, tag="u_sb",
                                    name=f"u{j}_{dv}", bufs=3)
                    nc.vector.tensor_copy(u_sb, ps_av)
                    dst = out[j * P:(j + 1) * P, dv * 512:(dv + 1) * 512]
                    if accum:
                        nc.gpsimd.dma_start(dst, u_sb,
                                            accum_op=mybir.AluOpType.add)
                    else:
                        nc.sync.dma_start(dst, u_sb)

    nc.compile()
    return nc


def _get_nc(reps=1, accum=False):
    key = ("nc", reps, accum)
    if key not in _cache:
        _cache[key] = _build_module(reps, accum)
    return _cache[key]


def make_in_maps(x, wq, wk, wv):
    import ml_dtypes
    bf = ml_dtypes.bfloat16
    x = np.asarray(x, np.float32)
    mt = ((np.asarray(wk, np.float64).T @ np.asarray(wq, np.float64))
          / np.sqrt(float(D))).astype(np.float32)
    wvT = np.ascontiguousarray(np.asarray(wv, np.float32).T)
    tri = np.triu(np.ones((P, P), np.float32))
    masks = {
        0: (tri, np.zeros((P, P), np.float32)),   # h=0: diag block, zero block
        1: (np.ones((P, P), np.float32), tri),    # h=1: ones block, diag block
    }
    in_maps = []
    for core in range(NCORES):
        b, h = core // 2, core % 2
        xTb = np.ascontiguousarray(x[b].T)               # [D, S]
        cols = np.concatenate(
            [np.arange((2 * j + h) * P, (2 * j + h + 1) * P) for j in range(NJ)])
        xtk = np.ascontiguousarray(xTb[:, cols])         # [D, SK]
        m0, m1 = masks[h]
        m = {
            "xT": xTb, "xtk": xtk, "mt": mt, "wvT": wvT,
            "mask0": m0, "mask1": m1, "onesd": np.ones((P, 1), np.float32),
        }
        in_maps.append({k: v.astype(bf) for k, v in m.items()})
    return in_maps


def gather(results):
    full = np.empty((B, S, D), np.float32)
    for core in range(NCORES):
        b, h = core // 2, core % 2
        o = results[core]["out"] / results[core]["cso"][0][:, None]
        for j in range(NJ):
            full[b, (2 * j + h) * P:(2 * j + h + 1) * P, :] = \
                o[j * P:(j + 1) * P, :]
    return full


def kernel(x, wq, wk, wv):
    from concourse.bass_utils import run_bass_kernel_spmd
    nc = _get_nc()
    in_maps = make_in_maps(x, wq, wk, wv)
    res = run_bass_kernel_spmd(nc, in_maps, core_ids=list(range(NCORES)))
    return gather(res.results)
